# revision 4
# baseline (speedup 1.0000x reference)
"""Trainium2 Bass kernel for nn_Attn_30623116820602.

Low-rank-projected causal multi-head attention:
  q/k/v = (x @ A) @ B  (rank 192), RoPE on q,k, causal softmax attention,
  output projection.  x: [128, 256, 768] fp32.

Sharding: pure data-parallel over batch (16 items per core, 8 cores).
Device layout is feature-major (d_model on partitions) throughout, so no
on-chip transposes are needed; the host pre-transposes x per core and
post-transposes the result.  RoPE's rotate-half is pre-folded into extra
weight matrices (qB_rot/kB_rot) so the tensor engine emits both q and
rot(q); softmax runs with keys on partitions (no max subtraction --
scores are ~N(0,1)), denominators come from ones-vector matmuls, and the
per-query normalization is broadcast via gpsimd and fused into the
PSUM->SBUF copy of the attention output.
"""

import math
import sys

sys.path.insert(0, "/opt/trn_rl_repo")

import numpy as np

B, T, D = 128, 256, 768
H, HD = 6, 128
RANK = 192  # padded to 256 on host
N_CORES = 8
B_LOC = B // N_CORES  # 16
N_PAIRS = B_LOC // 2  # 8 (2 batch items per pipeline iteration)
SCALE = 1.0 / math.sqrt(HD)

_CACHE = {}


def build_program(n_pairs=N_PAIRS):
    import concourse.tile as tile
    from concourse import bacc, mybir
    from contextlib import ExitStack

    f32 = mybir.dt.float32
    f32r = mybir.dt.float32r
    TOK = n_pairs * 512


    nc = bacc.Bacc("TRN2", target_bir_lowering=False, debug=False,
                   num_devices=N_CORES)

    def din(name, shape):
        return nc.dram_tensor(name, shape, f32, kind="ExternalInput").ap()

    xT = din("xT", [6, 128, TOK])
    qA_l, kA_l, vA_l = (din(n, [6, 128, 256]) for n in ("qA_l", "kA_l", "vA_l"))
    qB_l, qBr_l, kB_l, kBr_l, vB_l = (
        din(n, [2, 128, 768]) for n in ("qB_l", "qBr_l", "kB_l", "kBr_l", "vB_l"))
    ow_l = din("ow_l", [6, 128, 768])
    cos2 = din("cos2", [128, 512])
    sin2 = din("sin2", [128, 512])
    mask = din("mask", [128, 512])
    onec = din("onec", [128, 1])
    outT = nc.dram_tensor("outT", [6, 128, TOK], f32, kind="ExternalOutput").ap()

    with tile.TileContext(nc) as tc:
        with ExitStack() as ctx:
            wp = ctx.enter_context(tc.tile_pool(name="w", bufs=1))
            xp = ctx.enter_context(tc.tile_pool(name="xt", bufs=2))
            xrp = ctx.enter_context(tc.tile_pool(name="xr", bufs=1))
            qkp = ctx.enter_context(tc.tile_pool(name="qk", bufs=1))
            tp = ctx.enter_context(tc.tile_pool(name="tmp", bufs=2))
            ep = ctx.enter_context(tc.tile_pool(name="eexp", bufs=3))
            dp = ctx.enter_context(tc.tile_pool(name="den", bufs=1))
            bp = ctx.enter_context(tc.tile_pool(name="bcast", bufs=1))
            aop = ctx.enter_context(tc.tile_pool(name="ao", bufs=1))
            fp = ctx.enter_context(tc.tile_pool(name="fout", bufs=3))
            ps = ctx.enter_context(tc.tile_pool(name="ps", bufs=8, space="PSUM"))

            def psum():
                return ps.tile([128, 512], f32, tag="ps", name="psb")

            # ---- resident weights / constants ----
            def wload(name, src, shape, perm):
                t = wp.tile(shape, f32r, tag=name, name=name)
                nc.sync.dma_start(t[:], src.rearrange(perm).bitcast(f32r))
                return t

            qA_s = wload("qA", qA_l, [128, 6, 256], "k p m -> p k m")
            kA_s = wload("kA", kA_l, [128, 6, 256], "k p m -> p k m")
            vA_s = wload("vA", vA_l, [128, 6, 256], "k p m -> p k m")
            qB_s = wload("qB", qB_l, [128, 2, 768], "k p m -> p k m")
            qBr_s = wload("qBr", qBr_l, [128, 2, 768], "k p m -> p k m")
            kB_s = wload("kB", kB_l, [128, 2, 768], "k p m -> p k m")
            kBr_s = wload("kBr", kBr_l, [128, 2, 768], "k p m -> p k m")
            vB_s = wload("vB", vB_l, [128, 2, 768], "k p m -> p k m")
            ow_s = wload("ow", ow_l, [128, 6, 768], "k p m -> p k m")
            cos_s = wp.tile([128, 512], f32, tag="cos", name="cos")
            nc.sync.dma_start(cos_s[:], cos2)
            sin_s = wp.tile([128, 512], f32, tag="sin", name="sin")
            nc.sync.dma_start(sin_s[:], sin2)
            mask_s = wp.tile([128, 512], f32, tag="mask", name="mask")
            nc.sync.dma_start(mask_s[:], mask)
            onec_s = wp.tile([128, 1], f32r, tag="onec", name="onec")
            nc.sync.dma_start(onec_s[:], onec.bitcast(f32r))

            for pr in range(n_pairs):
                tok = slice(pr * 512, (pr + 1) * 512)
                # ---- load x^T for this pair of batch items ----
                xt = xp.tile([128, 6, 512], f32r, tag="xt", name="xt")
                nc.sync.dma_start(xt[:], xT[:, :, tok].rearrange("k p t -> p k t").bitcast(f32r))

                # ---- proj1: xr^T = A^T @ x^T  (rank padded to 256) ----
                xrs = {}
                for pname, A_s in (("q", qA_s), ("k", kA_s), ("v", vA_s)):
                    mm = [psum(), psum()]
                    for mt in range(2):
                        for kt in range(6):
                            nc.tensor.matmul(
                                mm[mt][:],
                                A_s[:, kt, mt * 128:(mt + 1) * 128],
                                xt[:, kt, :],
                                start=(kt == 0), stop=(kt == 5))
                    xr = xrp.tile([128, 2, 512], f32r, tag=f"xr_{pname}", name=f"xr_{pname}")
                    nc.scalar.copy(xr[:, 0, :], mm[0][:])
                    nc.scalar.copy(xr[:, 1, :], mm[1][:])
                    xrs[pname] = xr

                # ---- proj2 + RoPE for q and k (feature-major) ----
                qk = {}
                for pname, B_s, Br_s in (("q", qB_s, qBr_s), ("k", kB_s, kBr_s)):
                    sb = qkp.tile([128, 3072], f32r, tag=f"{pname}sb", name=f"{pname}sb")
                    xr = xrs[pname]
                    for h in range(H):
                        hs = slice(h * 512, (h + 1) * 512)
                        p_main = psum()
                        p_rot = psum()
                        for kt in range(2):
                            nc.tensor.matmul(
                                p_main[:],
                                B_s[:, kt, h * 128:(h + 1) * 128],
                                xr[:, kt, :],
                                start=(kt == 0), stop=(kt == 1))
                        for kt in range(2):
                            nc.tensor.matmul(
                                p_rot[:],
                                Br_s[:, kt, h * 128:(h + 1) * 128],
                                xr[:, kt, :],
                                start=(kt == 0), stop=(kt == 1))
                        tmp = tp.tile([128, 512], f32, tag="ropetmp", name="ropetmp")
                        nc.vector.tensor_tensor(
                            sb[:, hs], p_main[:], cos_s[:], mybir.AluOpType.mult)
                        nc.vector.tensor_tensor(
                            tmp[:], p_rot[:], sin_s[:], mybir.AluOpType.mult)
                        nc.vector.tensor_tensor(
                            sb[:, hs], sb[:, hs].bitcast(f32), tmp[:],
                            mybir.AluOpType.add)
                    qk[pname] = sb
                qsb, ksb = qk["q"], qk["k"]

                # ---- proj2 for v (token-major) ----
                vsb = qkp.tile([128, 4, 768], f32r, tag="vsb", name="vsb")
                xrv = xrs["v"]
                for mt in range(4):
                    for nch in range(2):
                        vp = psum()
                        for kt in range(2):
                            nc.tensor.matmul(
                                vp[:, 0:384],
                                xrv[:, kt, mt * 128:(mt + 1) * 128],
                                vB_s[:, kt, nch * 384:(nch + 1) * 384],
                                start=(kt == 0), stop=(kt == 1))
                        nc.scalar.copy(vsb[:, mt, nch * 384:(nch + 1) * 384],
                                       vp[:, 0:384])

                # ---- attention (per batch item, per head) ----
                aosb = aop.tile([128, 6, 512], f32r, tag="aosb", name="aosb")
                for b in range(2):
                    d_sb = dp.tile([1, 1536], f32, tag="dsb", name="dsb")
                    o_group = []
                    for h in range(H):
                        qcol = slice(h * 512 + b * 256, h * 512 + b * 256 + 256)
                        sp = psum()
                        for kt in range(2):
                            nc.tensor.matmul(
                                sp[:, kt * 256:(kt + 1) * 256],
                                ksb[:, h * 512 + b * 256 + kt * 128:
                                      h * 512 + b * 256 + kt * 128 + 128],
                                qsb[:, qcol],
                                start=True, stop=True)
                        E0 = ep.tile([128, 512], f32, tag="E0", name="E0")
                        nc.scalar.activation(
                            E0[:], sp[:], mybir.ActivationFunctionType.Exp,
                            scale=SCALE)
                        E = ep.tile([128, 512], f32r, tag="E", name="E")
                        nc.vector.tensor_tensor(
                            E[:], E0[:], mask_s[:], mybir.AluOpType.mult)
                        dps = psum()
                        for kt in range(2):
                            nc.tensor.matmul(
                                dps[0:1, 0:256], onec_s[:],
                                E[:, kt * 256:(kt + 1) * 256],
                                start=(kt == 0), stop=(kt == 1))
                        nc.scalar.copy(d_sb[0:1, h * 256:(h + 1) * 256],
                                       dps[0:1, 0:256])
                        if h % 2 == 0:
                            o_group.append(psum())
                        ops_t = o_group[h // 2]
                        for kt in range(2):
                            nc.tensor.matmul(
                                ops_t[:, (h % 2) * 256:(h % 2) * 256 + 256],
                                vsb[:, b * 2 + kt, h * 128:(h + 1) * 128],
                                E[:, kt * 256:(kt + 1) * 256],
                                start=(kt == 0), stop=(kt == 1))
                    # normalize all 6 heads of this batch item
                    invd = dp.tile([1, 1536], f32, tag="invd", name="invd")
                    nc.vector.reciprocal(invd[:], d_sb[:])
                    bD = bp.tile([128, 1536], f32, tag="bD", name="bD")
                    nc.gpsimd.partition_broadcast(bD[:], invd[:])
                    for g in range(3):
                        nc.vector.tensor_tensor(
                            aosb[:, 2 * g:2 * g + 2, b * 256:(b + 1) * 256],
                            o_group[g][:].rearrange("p (h t) -> p h t", h=2),
                            bD[:, g * 512:(g + 1) * 512].rearrange(
                                "p (h t) -> p h t", h=2),
                            mybir.AluOpType.mult)

                # ---- output projection ----
                for mt in range(6):
                    fps = psum()
                    for kt in range(6):
                        nc.tensor.matmul(
                            fps[:],
                            ow_s[:, kt, mt * 128:(mt + 1) * 128],
                            aosb[:, kt, :],
                            start=(kt == 0), stop=(kt == 5))
                    fout = fp.tile([128, 512], f32, tag="fout", name="fout")
                    nc.scalar.copy(fout[:], fps[:])
                    nc.sync.dma_start(outT[mt, :, tok], fout[:])

    nc.compile()
    return nc


def _rope_tables():
    inv = 1.0 / (10000.0 ** (np.arange(0, HD, 2, dtype=np.float32) / HD))
    t = np.arange(T, dtype=np.float32)
    freqs = np.outer(t, inv)                      # [T, 64]
    emb = np.concatenate([freqs, freqs], axis=-1)  # [T, 128]
    return np.cos(emb).astype(np.float32), np.sin(emb).astype(np.float32)


def _prep_shared(qA, qB, kA, kB, vA, vB, o_w):
    """Host-side weight/constant layouts (shared by all cores)."""
    def a_layout(A):  # [768,192] -> pad to [768,256] -> [6,128,256]
        Ap = np.zeros((D, 256), np.float32)
        Ap[:, :RANK] = A
        return np.ascontiguousarray(Ap.reshape(6, 128, 256))

    def b_layout(Bm):  # [192,768] -> pad to [256,768] -> [2,128,768]
        Bp = np.zeros((256, D), np.float32)
        Bp[:RANK] = Bm
        return np.ascontiguousarray(Bp.reshape(2, 128, D))

    def rot_weights(Bm):  # fold rotate-half into the projection weights
        Br = np.empty_like(Bm)
        for h in range(H):
            c = h * HD
            Br[:, c:c + 64] = -Bm[:, c + 64:c + 128]
            Br[:, c + 64:c + 128] = Bm[:, c:c + 64]
        return Br

    cos, sin = _rope_tables()
    cosT = np.ascontiguousarray(cos.T)  # [128, 256]
    sinT = np.ascontiguousarray(sin.T)
    cos2 = np.concatenate([cosT, cosT], axis=1)  # [128, 512] (2 batch items)
    sin2 = np.concatenate([sinT, sinT], axis=1)

    p = np.arange(128)[:, None]
    t = np.arange(T)[None, :]
    m0 = (p <= t).astype(np.float32)          # keytile 0
    m1 = (p + 128 <= t).astype(np.float32)    # keytile 1
    mask = np.concatenate([m0, m1], axis=1)   # [128, 512]

    return {
        "qA_l": a_layout(qA), "kA_l": a_layout(kA), "vA_l": a_layout(vA),
        "qB_l": b_layout(qB), "qBr_l": b_layout(rot_weights(qB)),
        "kB_l": b_layout(kB), "kBr_l": b_layout(rot_weights(kB)),
        "vB_l": b_layout(vB),
        "ow_l": np.ascontiguousarray(o_w.reshape(6, 128, D)),
        "cos2": cos2, "sin2": sin2, "mask": mask,
        "onec": np.ones((128, 1), np.float32),
    }


def x_to_xT(xc):
    """[b, T, D] -> [6, 128, b*T] feature-major, batch-major tokens."""
    nb = xc.shape[0]
    return np.ascontiguousarray(
        xc.reshape(nb, T, 6, 128).transpose(2, 3, 0, 1).reshape(6, 128, nb * T))


def outT_to_out(oT, nb):
    return np.ascontiguousarray(
        oT.reshape(6, 128, nb, T).transpose(2, 3, 0, 1).reshape(nb, T, D))


def kernel(x, qA, qB, kA, kB, vA, vB, o_w):
    from concourse import bass_utils

    if "nc" not in _CACHE:
        _CACHE["nc"] = build_program(N_PAIRS)
    nc = _CACHE["nc"]

    shared = _prep_shared(
        np.asarray(qA, np.float32), np.asarray(qB, np.float32),
        np.asarray(kA, np.float32), np.asarray(kB, np.float32),
        np.asarray(vA, np.float32), np.asarray(vB, np.float32),
        np.asarray(o_w, np.float32))
    x = np.asarray(x, np.float32)

    in_maps = []
    for c in range(N_CORES):
        m = dict(shared)
        m["xT"] = x_to_xT(x[c * B_LOC:(c + 1) * B_LOC])
        in_maps.append(m)

    res = bass_utils.run_bass_kernel_spmd(
        nc, in_maps, core_ids=list(range(N_CORES)))
    out = np.empty((B, T, D), np.float32)
    for c in range(N_CORES):
        out[c * B_LOC:(c + 1) * B_LOC] = outT_to_out(
            res.results[c]["outT"], B_LOC)
    return out
